# revision 2
# baseline (speedup 1.0000x reference)
"""DCT-II embedding kernel for Trainium2 (8 NeuronCores, data parallel over batch).

Computes out[b,k,j,c] = sum_n C[k,n] * x[b,n,j,c] with C the (unnormalized,
scaled-by-2) DCT-II cosine basis, for x of shape (8192, 100, 32, 3) fp32.

Sharding: pure data parallel — batch axis split 8 ways; the 100x100 basis is
replicated (baked into per-core weight inputs).

Layouts:
  * "slab2": HBM<->SBUF DMAs move contiguous 768B runs (2 consecutive
    (b,n)-rows per partition).  The DCT becomes 4 block-diagonal 100x100
    weight matmuls per 2-batch block, accumulated in PSUM.
  * "straight": partition dim = n directly; 1 matmul per 4-batch group but
    384B strided DMA runs.
"""

import numpy as np

import concourse.bacc as bacc
import concourse.mybir as mybir
from concourse.tile import TileContext
from concourse.bass_utils import run_bass_kernel_spmd

N_CORES = 8
B_FULL = 8192
B_CORE = B_FULL // N_CORES   # 1024
N = 100                      # DCT length (axis 1)
M = 96                       # 32*3 flattened inner dims
ROWS_CORE = B_CORE * N       # 102400 rows of 96 floats per core

# ---------------------------------------------------------------- weights


def _dct_matrix() -> np.ndarray:
    n = np.arange(N)
    k = np.arange(N)[:, None]
    return (2.0 * np.cos(np.pi * (2.0 * n[None, :] + 1.0) * k / (2.0 * N))).astype(
        np.float32
    )


def _slab_weights() -> np.ndarray:
    """W[2*s+sp][p,q] = C[k(q,sp), n(p,s)] on the matching 50-row half, else 0.

    Partition p of an input block holds x rows 2p+s (s in {0,1}); partition q
    of an output block holds out rows 2q+sp.  Rows 0..99 of a 200-row block
    are batch b0 (partitions 0..49), rows 100..199 are b1 (partitions 50..99).
    """
    C = _dct_matrix()
    W = np.zeros((4, N, N), np.float32)
    i = np.arange(50)
    for s in (0, 1):
        for sp in (0, 1):
            blk = C[np.ix_(2 * i + sp, 2 * i + s)].T  # [p_half, q_half]
            for h in (0, 1):
                W[2 * s + sp, 50 * h : 50 * h + 50, 50 * h : 50 * h + 50] = blk
    return W


# ---------------------------------------------------------------- builders


def build(layout="slab2", use_f32r=True, repeat=1):
    """Build the per-core Bass program.  Returns (nc, static_inputs)."""
    dt_in = mybir.dt.float32r if use_f32r else mybir.dt.float32
    nc = bacc.Bacc("TRN2", target_bir_lowering=False, debug=False)

    x = nc.dram_tensor("x", [ROWS_CORE, M], dt_in, kind="ExternalInput")
    y = nc.dram_tensor("y", [ROWS_CORE, M], mybir.dt.float32, kind="ExternalOutput")

    if layout == "slab2":
        w = nc.dram_tensor("w", [4, N, N], dt_in, kind="ExternalInput")
        static = {"w": _slab_weights()}
    else:
        w = nc.dram_tensor("w", [N, N], dt_in, kind="ExternalInput")
        static = {"w": np.ascontiguousarray(_dct_matrix().T)}  # ct[n,k]

    with TileContext(nc) as tc:
        with (
            tc.tile_pool(name="wpool", bufs=1) as wpool,
            tc.tile_pool(name="inpool", bufs=3) as inpool,
            tc.tile_pool(name="outpool", bufs=3) as outpool,
            tc.tile_pool(name="psum", bufs=6, space="PSUM") as pspool,
        ):
            if layout == "slab2":
                wt = wpool.tile([N, 4 * N], dt_in)
                nc.sync.dma_start(
                    out=wt[:].rearrange("p (w q) -> p w q", w=4),
                    in_=w[:].rearrange("w p q -> p w q"),
                )
                body = lambda: _slab2_body(nc, tc, x, y, wt, inpool, outpool, pspool, dt_in)
            else:
                wt = wpool.tile([N, N], dt_in)
                nc.sync.dma_start(out=wt[:], in_=w[:])
                body = lambda: _straight_body(nc, tc, x, y, wt, inpool, outpool, pspool, dt_in)

            if repeat == 1:
                body()
            else:
                with tc.For_i(0, repeat, 1):
                    body()

    nc.compile()
    return nc, static


def _slab2_body(nc, tc, x, y, wt, inpool, outpool, pspool, dt_in):
    NBLK = 16          # 200-row blocks per megatile (32 batches)
    GRP = 4            # matmul groups per megatile
    TBLK = NBLK // GRP  # blocks per matmul group -> free dim 4*96 = 384
    ROWS_TILE = 200 * NBLK
    n_tiles = ROWS_CORE // ROWS_TILE  # 32

    x_blk = x[:].rearrange("(t blk p s) m -> t p blk (s m)", p=N, s=2, blk=NBLK)
    y_blk = y[:].rearrange("(t blk p s) m -> t p blk (s m)", p=N, s=2, blk=NBLK)

    for t in range(n_tiles):
        in_t = inpool.tile([N, NBLK * 192], dt_in)
        nc.sync.dma_start(
            out=in_t[:].rearrange("p (blk sm) -> p blk sm", blk=NBLK),
            in_=x_blk[t],
        )
        out_t = outpool.tile([N, NBLK * 192], mybir.dt.float32)
        in_v = in_t[:].rearrange(
            "p (grp blk s m) -> p grp s blk m", grp=GRP, blk=TBLK, s=2, m=M
        )
        out_v = out_t[:].rearrange(
            "p (grp blk s m) -> p grp s blk m", grp=GRP, blk=TBLK, s=2, m=M
        )
        for g in range(GRP):
            for sp in (0, 1):
                ps = pspool.tile([N, TBLK * M], mybir.dt.float32)
                for s in (0, 1):
                    nc.tensor.matmul(
                        ps[:],
                        lhsT=wt[:, (2 * s + sp) * N : (2 * s + sp + 1) * N],
                        rhs=in_v[:, g, s],
                        start=(s == 0),
                        stop=(s == 1),
                    )
                src = ps[:].rearrange("p (blk m) -> p blk m", blk=TBLK)
                dst = out_v[:, g, sp]
                if (g + sp) % 2 == 0:
                    nc.scalar.copy(out=dst, in_=src)
                else:
                    nc.vector.tensor_copy(dst, src)
        nc.sync.dma_start(
            out=y_blk[t],
            in_=out_t[:].rearrange("p (blk sm) -> p blk sm", blk=NBLK),
        )


def _straight_body(nc, tc, x, y, wt, inpool, outpool, pspool, dt_in):
    NB = 32            # batches per megatile
    GRP = 8            # matmul groups per megatile
    TB = NB // GRP     # batches per group -> free dim 4*96 = 384
    n_tiles = B_CORE // NB  # 32

    x_b = x[:].rearrange("(t b n) m -> t n b m", n=N, b=NB)
    y_b = y[:].rearrange("(t b n) m -> t n b m", n=N, b=NB)

    for t in range(n_tiles):
        in_t = inpool.tile([N, NB * M], dt_in)
        nc.sync.dma_start(
            out=in_t[:].rearrange("p (b m) -> p b m", b=NB), in_=x_b[t]
        )
        out_t = outpool.tile([N, NB * M], mybir.dt.float32)
        for g in range(GRP):
            ps = pspool.tile([N, TB * M], mybir.dt.float32)
            nc.tensor.matmul(
                ps[:],
                lhsT=wt[:],
                rhs=in_t[:, g * TB * M : (g + 1) * TB * M],
                start=True,
                stop=True,
            )
            dst = out_t[:, g * TB * M : (g + 1) * TB * M]
            if g % 2 == 0:
                nc.scalar.copy(out=dst, in_=ps[:])
            else:
                nc.vector.tensor_copy(dst, ps[:])
        nc.sync.dma_start(
            out=y_b[t], in_=out_t[:].rearrange("p (b m) -> p b m", b=NB)
        )


# ---------------------------------------------------------------- entry point

_CACHE = {}


def _get_program(layout="slab2", use_f32r=True, repeat=1):
    key = (layout, use_f32r, repeat)
    if key not in _CACHE:
        _CACHE[key] = build(layout, use_f32r, repeat)
    return _CACHE[key]


def kernel(x: np.ndarray) -> np.ndarray:
    assert x.shape == (B_FULL, N, 32, 3) and x.dtype == np.float32
    nc, static = _get_program()
    xs = np.ascontiguousarray(x).reshape(N_CORES, ROWS_CORE, M)
    in_maps = [{"x": xs[i], **static} for i in range(N_CORES)]
    res = run_bass_kernel_spmd(nc, in_maps, core_ids=list(range(N_CORES)))
    out = np.stack([r["y"] for r in res.results])
    return out.reshape(B_FULL, N, 32, 3)
